# revision 25
# baseline (speedup 1.0000x reference)
"""Trainium2 Bass kernel for nn_BMEDModel (sequential 10k-step Euler ODE with a
tiny MLP migration model).

The recurrence is strictly sequential (each Euler step's MLP input is the
previous step's concentrations), so the work runs on one NeuronCore
(replicated across all 8 for the SPMD contract) with every weight resident in
SBUF and the whole trajectory accumulated in SBUF; DRAM is touched only at the
start (weights) and end (trajectory readout).

Per step, on device (col-major layout, hidden H=192 split 96+96):
    z1 = [C;1] @ [W1v;b1]      2 matmuls (K=5, M=96 each) -> PSUM [96,2]
    h1 = relu(z1)              DVE, PSUM->SBUF
    z2 = [h1;1] @ [W2;b2]      4 matmuls (K=96+97, M=96+96)
    ... z3 likewise ...
    acc += [h3;1] @ W4ext      2 matmuls, start=False: PSUM keeps the running
                               sums num[lanes 0-3] / den & volumes[lanes 32-35]
                               (algebra: C_t*V_t == num_{t-1} exactly, so the
                               state update reduces to accumulating +-dN*dt)
    rden = 1/den               DVE reciprocal (PSUM->SBUF)
    x'   = num * rden          DVE, written to the fp16 MLP-input ping-pong
    trajC= num * rden (fp32)   ScalarE ACT(Copy, scale=rden), off critical path
    trajV= den copy   (fp32)   ScalarE, off critical path

The two M-chunks of each layer write adjacent PSUM banks of one tile so their
accumulation groups (start/stop) stay independent while a single strided DVE
relu evacuates both.

Matmul operands are fp16 (weights quantized ~5e-4) because fp32 matmuls lower
to a LOW/HIGH double-pass (2x LDWEIGHTS+MATMUL); the trajectory itself is
computed and stored in fp32.  The reference trajectory is chaotic (volumes
cross zero ~20x; even fp64 vs the fp32 reference ends at O(1) relative error),
so fidelity is bounded by the pre-chaos prefix, where fp16 weights track the
reference to ~2e-4 (fp32: ~2e-5) against a state scale of ~1e0-1e5.

Host-side prep only folds constants: mean/scale into W1, dt and the state
update's +-1 scatter into W4, biases as an appended all-ones input row, and
the T/V/E components (constant throughout the reference trajectory) into b1.
"""

import os

import numpy as np

DT = 0.1
H = 192
P0 = 96  # first K/M chunk size (H = 2*P0)


def _fold_weights(init_state, times, mean, scale, W1, b1, W2, b2, W3, b3, W4, b4):
    f32 = np.float32
    s0 = np.asarray(init_state, f32)[0]
    mean = np.asarray(mean, f32)
    scale = np.asarray(scale, f32)
    W1 = np.asarray(W1, f32)
    b1 = np.asarray(b1, f32)

    # x = (s - mean)/scale fed to W1  ==>  z1 = s @ (W1/scale) + (b1 - (mean/scale)@W1)
    W1eff = W1 / scale[:, None]
    b1eff = b1 - (mean / scale) @ W1
    # state[0:3] (T, V, E) never changes; fold its layer-1 contribution into the bias
    b1full = b1eff + s0[0:3] @ W1eff[0:3]
    W1v = W1eff[3:7]  # [4, H], input = raw concentrations
    lhs1 = np.concatenate([W1v, b1full[None]], axis=0).astype(f32)  # [5, H]

    lhs2 = np.concatenate([np.asarray(W2, f32), np.asarray(b2, f32)[None]], axis=0)
    lhs3 = np.concatenate([np.asarray(W3, f32), np.asarray(b3, f32)[None]], axis=0)

    # Extended head: 36 outputs (engine APs need 32-aligned partition bases):
    # lanes 0-3 = num increments, lanes 32-35 = den/volume increments
    # (den = [nVF, nVA, nVF, nVB]; the volume triple is lanes [32, 33, 35]).
    # mig = [dNLA, dNK, dVA, dVB] (rates); dN = mig*dt
    A = np.zeros((4, 36), f32)
    A[0, 0] = -1.0  # num CF_LA: -dNLA
    A[0, 1] = +1.0  # num CA_LA: +dNLA
    A[1, 2] = -1.0  # num CF_K:  -dNK
    A[1, 3] = +1.0  # num CB_K:  +dNK
    for j in (32, 34):  # VF lanes: -(dVA+dVB)
        A[2, j] = -1.0
        A[3, j] = -1.0
    A[2, 33] = +1.0  # VA: +dVA
    A[3, 35] = +1.0  # VB: +dVB
    W4e = (np.asarray(W4, f32) @ A) * f32(DT)  # [H, 36]
    b4e = (np.asarray(b4, f32) @ A) * f32(DT)  # [36]
    lhs4 = np.concatenate([W4e, b4e[None]], axis=0).astype(f32)  # [H+1, 36]

    C0 = s0[3:7]
    vden0 = s0[[7, 8, 7, 9]]  # [VF, VA, VF, VB]
    num0 = (C0 * vden0).astype(f32)
    init36 = np.zeros((1, 36), f32)
    init36[0, 0:4] = num0
    init36[0, 32:36] = vden0

    mode = os.environ.get("BMED_MMDT", "f16w")
    wnp = np.float16 if mode == "f16w" else f32
    tensors = {
        "lhs1": np.ascontiguousarray(lhs1.astype(wnp)),
        "l2k0": np.ascontiguousarray(lhs2[0:P0].astype(wnp)),
        "l2k1": np.ascontiguousarray(lhs2[P0:].astype(wnp)),
        "l3k0": np.ascontiguousarray(lhs3[0:P0].astype(wnp)),
        "l3k1": np.ascontiguousarray(lhs3[P0:].astype(wnp)),
        "l4k0": np.ascontiguousarray(lhs4[0:P0].astype(wnp)),
        "l4k1": np.ascontiguousarray(lhs4[P0:].astype(wnp)),
        "init36": np.ascontiguousarray(init36),
        "initC": np.ascontiguousarray(C0[:, None]),              # fp32, traj col 0
        "initX": np.ascontiguousarray(C0[:, None].astype(wnp)),  # MLP input col 0
        "initV": np.ascontiguousarray(vden0[:, None]),
    }
    return tensors, s0


def _build(nc, tc, n_steps, unroll):
    from contextlib import ExitStack

    import concourse.mybir as mybir
    from concourse.bass import ds

    f32 = mybir.dt.float32
    mode = os.environ.get("BMED_MMDT", "f16w")
    wdt = mybir.dt.float16 if mode == "f16w" else f32  # matmul operand dtype
    Kb = P0 + 1  # 97: weight rows P0..191 plus the bias row

    ins = {}
    for name, shape, dt_ in [
        ("lhs1", (5, H), wdt),
        ("l2k0", (P0, H), wdt), ("l2k1", (Kb, H), wdt),
        ("l3k0", (P0, H), wdt), ("l3k1", (Kb, H), wdt),
        ("l4k0", (P0, 36), wdt), ("l4k1", (Kb, 36), wdt),
        ("init36", (1, 36), f32),
        ("initC", (4, 1), f32), ("initX", (4, 1), wdt), ("initV", (4, 1), f32),
    ]:
        ins[name] = nc.dram_tensor(name, shape, dt_, kind="ExternalInput").ap()

    outC = nc.dram_tensor("outC", (4, n_steps + 1), f32, kind="ExternalOutput").ap()
    outV = nc.dram_tensor("outV", (4, n_steps + 1), f32, kind="ExternalOutput").ap()

    n_blocks = n_steps // unroll
    assert n_blocks * unroll == n_steps
    assert unroll % 2 == 0  # x16 ping-pong parity

    stack = ExitStack()
    const = stack.enter_context(tc.tile_pool(name="const", bufs=1))
    psum = stack.enter_context(tc.tile_pool(name="psum", bufs=3, space="PSUM"))
    accp = stack.enter_context(tc.tile_pool(name="accp", bufs=1, space="PSUM"))

    # Weights, resident in SBUF for the whole kernel
    w = {}
    for name, shape in [
        ("lhs1", (5, H)),
        ("l2k0", (P0, H)), ("l2k1", (Kb, H)),
        ("l3k0", (P0, H)), ("l3k1", (Kb, H)),
        ("l4k0", (P0, 36)), ("l4k1", (Kb, 36)),
    ]:
        w[name] = const.tile(list(shape), wdt, name=f"w_{name}", tag=name)
        nc.sync.dma_start(out=w[name][:, :], in_=ins[name])
    w_init36 = const.tile([1, 36], f32, name="w_init36", tag="init36")
    nc.sync.dma_start(out=w_init36[:, :], in_=ins["init36"])

    # Output trajectories (fp32)
    trajC = const.tile([4, n_steps + 1], f32, name="trajC", tag="trajC")
    trajV = const.tile([4, n_steps + 1], f32, name="trajV", tag="trajV")
    nc.sync.dma_start(out=trajC[0:4, 0:1], in_=ins["initC"])
    nc.sync.dma_start(out=trajV[0:4, 0:1], in_=ins["initV"])

    # MLP input ping-pong: [C(4); 1.0] per column; row 4 = constant 1 for bias
    x16 = const.tile([5, 2], wdt, name="x16", tag="x16")
    nc.vector.memset(x16[0:5, :], 1.0)
    nc.sync.dma_start(out=x16[0:4, 0:1], in_=ins["initX"])

    one11 = const.tile([1, 1], f32, name="one11", tag="one11")
    nc.vector.memset(one11[:, :], 1.0)
    rden = const.tile([4, 1], f32, name="rden", tag="rden")
    stage = const.tile([36, 1], f32, name="stage", tag="stage")

    # Hidden tiles [97, 2]; (96, col1) holds the constant 1.0 for bias rows
    hids = []
    for li in range(3):
        ht = const.tile([Kb, 2], wdt, name=f"h{li}", tag=f"h{li}")
        nc.vector.memset(ht[P0:Kb, 1:2], 1.0)
        hids.append(ht)

    # Running sums live in PSUM: lanes 0-3 = num, lanes 32-35 = den/vols
    acc = accp.tile([36, 1], f32, name="acc", tag="acc")
    nc.tensor.matmul(acc[:, :], w_init36[:, :], one11[:, :],
                     start=True, stop=False, skip_group_check=True)

    use_stage = os.environ.get("BMED_STAGE", "0") == "1"
    BANKF = 512  # fp32 elements per PSUM bank (per partition)

    def zpair(z):
        # the two matmul outputs live in adjacent PSUM banks of one tile;
        # view them as [96, 2] with free-stride 512 for a single relu
        return z[:, :].rearrange("p (b c) -> p b c", c=BANKF)[:, :, 0:1]

    def step(u, t_next):
        rhs_x = x16[0:5, (u % 2):(u % 2) + 1]
        z1 = psum.tile([P0, 2 * BANKF], f32, name="z1", tag="z")
        nc.tensor.matmul(z1[:, 0:1], w["lhs1"][:, 0:P0], rhs_x, start=True, stop=True)
        nc.tensor.matmul(z1[:, BANKF:BANKF + 1], w["lhs1"][:, P0:H], rhs_x,
                         start=True, stop=True)
        nc.vector.tensor_relu(hids[0][0:P0, 0:2], zpair(z1))

        for li, (k0, k1) in enumerate((("l2k0", "l2k1"), ("l3k0", "l3k1"))):
            h_in, h_out = hids[li], hids[li + 1]
            z = psum.tile([P0, 2 * BANKF], f32, name="z", tag="z")
            # K1-major order: both K1 matmuls issue as soon as relu_a lands;
            # per-bank accumulation groups keep start/stop independent
            for m_ in range(2):
                nc.tensor.matmul(z[:, m_ * BANKF:m_ * BANKF + 1],
                                 w[k0][:, m_ * P0:(m_ + 1) * P0],
                                 h_in[0:P0, 0:1], start=True, stop=False)
            for m_ in range(2):
                nc.tensor.matmul(z[:, m_ * BANKF:m_ * BANKF + 1],
                                 w[k1][:, m_ * P0:(m_ + 1) * P0],
                                 h_in[0:Kb, 1:2], start=False, stop=True)
            nc.vector.tensor_relu(h_out[0:P0, 0:2], zpair(z))

        h3 = hids[2]
        nc.tensor.matmul(acc[:, :], w["l4k0"][:, :], h3[0:P0, 0:1],
                         start=False, stop=False, skip_group_check=True)
        nc.tensor.matmul(acc[:, :], w["l4k1"][:, :], h3[0:Kb, 1:2],
                         start=False, stop=False, skip_group_check=True)

        # the only acc readers are recip + mult, so the next step's
        # accumulate is not gated by the trajectory writes
        xcol = x16[0:4, ((u + 1) % 2):((u + 1) % 2) + 1]
        nc.vector.reciprocal(rden[:, :], acc[32:36, 0:1])
        nc.vector.tensor_tensor(xcol, acc[0:4, 0:1], rden[:, :],
                                op=mybir.AluOpType.mult)
        # off critical path: trajC = cast of the fp16 x (ScalarE);
        # trajV = 1/rden (DVE; double rounding ~1 ulp)
        nc.scalar.activation(trajC[0:4, t_next], xcol,
                             mybir.ActivationFunctionType.Copy)
        nc.vector.reciprocal(trajV[0:4, t_next], rden[:, :])

    if n_blocks > 1:
        with tc.For_i(0, n_steps, unroll) as i:
            for u in range(unroll):
                step(u, ds(i + u + 1, 1))
    else:
        for u in range(unroll):
            step(u, ds(u + 1, 1))

    if os.environ.get("BMED_DEBUG_H"):
        dbg = nc.dram_tensor("dbgH", (Kb, 6), mybir.dt.float32,
                             kind="ExternalOutput").ap()
        dbg_sb = const.tile([Kb, 6], f32, name="dbg_sb", tag="dbg_sb")
        for li in range(3):
            rows = Kb if li == 2 else P0
            nc.vector.tensor_copy(dbg_sb[0:rows, 2 * li:2 * li + 2],
                                  hids[li][0:rows, 0:2])
        nc.sync.dma_start(out=dbg, in_=dbg_sb[:, :])
    nc.sync.dma_start(out=outC, in_=trajC[0:4, :])
    nc.sync.dma_start(out=outV, in_=trajV[0:4, :])
    stack.close()


def _run_device(tensors, n_steps, unroll, trace=False):
    import concourse.bacc as bacc
    import concourse.bass_utils as bass_utils
    import concourse.tile as tile

    nc = bacc.Bacc("TRN2", target_bir_lowering=False, debug=False,
                   enable_asserts=False, num_devices=8)
    with tile.TileContext(nc) as tc:
        _build(nc, tc, n_steps, unroll)
    nc.compile()

    res = bass_utils.run_bass_kernel_spmd(
        nc, [dict(tensors) for _ in range(8)], core_ids=list(range(8)), trace=trace,
    )
    if getattr(kernel, "_keep_results", False):
        kernel._last_nc = nc
        kernel._last_tensors = tensors
    return res


def kernel(init_state, times, mean, scale, W1, b1, W2, b2, W3, b3, W4, b4):
    times = np.asarray(times)
    n_steps = int(round(float(times[0, -1]) / DT))
    n_steps_env = os.environ.get("BMED_NSTEPS")
    if n_steps_env is not None:
        n_steps = int(n_steps_env)
    unroll = int(os.environ.get("BMED_UNROLL", "20"))
    while n_steps % unroll or unroll % 2:
        unroll -= 1

    tensors, s0 = _fold_weights(init_state, times, mean, scale,
                                W1, b1, W2, b2, W3, b3, W4, b4)
    res = _run_device(tensors, n_steps, unroll,
                      trace=bool(int(os.environ.get("BMED_TRACE", "0"))))
    out = res.results[0]
    C = out["outC"]  # [4, n+1]
    V = out["outV"]  # [4, n+1] = [VF, VA, VF, VB]

    pred = np.empty((n_steps + 1, 10), np.float32)
    pred[:, 0:3] = s0[0:3]
    pred[:, 3:7] = C.T
    pred[:, 7:10] = V[[0, 1, 3], :].T

    measured_indices = np.concatenate([
        np.array([0], dtype=np.int32),
        np.round(times[0] / DT).astype(np.int32),
    ])
    if getattr(kernel, "_keep_results", False):
        kernel._last_res = res
    return pred, measured_indices


# revision 26
# speedup vs baseline: 1.1015x; 1.1015x over previous
"""Trainium2 Bass kernel for nn_BMEDModel (sequential 10k-step Euler ODE with a
tiny MLP migration model).

The recurrence is strictly sequential (each Euler step's MLP input is the
previous step's concentrations), so the work runs on one NeuronCore
(replicated across all 8 for the SPMD contract) with every weight resident in
SBUF and the whole trajectory accumulated in SBUF; DRAM is touched only at the
start (weights) and end (trajectory readout).

Per step, on device (col-major layout, hidden H=192 split 96+96):
    z1 = [C;1] @ [W1v;b1]      2 matmuls (K=5, M=96 each) -> PSUM [96,2]
    h1 = relu(z1)              DVE, PSUM->SBUF
    z2 = [h1;1] @ [W2;b2]      4 matmuls (K=96+97, M=96+96)
    ... z3 likewise ...
    acc += [h3;1] @ W4ext      2 matmuls, start=False: PSUM keeps the running
                               sums num[lanes 0-3] / den & volumes[lanes 32-35]
                               (algebra: C_t*V_t == num_{t-1} exactly, so the
                               state update reduces to accumulating +-dN*dt)
    rden = 1/den               DVE reciprocal (PSUM->SBUF)
    x'   = num * rden          DVE, written to the fp16 MLP-input ping-pong
    trajC= num * rden (fp32)   ScalarE ACT(Copy, scale=rden), off critical path
    trajV= den copy   (fp32)   ScalarE, off critical path

The two M-chunks of each layer write adjacent PSUM banks of one tile so their
accumulation groups (start/stop) stay independent while a single strided DVE
relu evacuates both.

Matmul operands are fp16 (weights quantized ~5e-4) because fp32 matmuls lower
to a LOW/HIGH double-pass (2x LDWEIGHTS+MATMUL); the trajectory itself is
computed and stored in fp32.  The reference trajectory is chaotic (volumes
cross zero ~20x; even fp64 vs the fp32 reference ends at O(1) relative error),
so fidelity is bounded by the pre-chaos prefix, where fp16 weights track the
reference to ~2e-4 (fp32: ~2e-5) against a state scale of ~1e0-1e5.

Host-side prep only folds constants: mean/scale into W1, dt and the state
update's +-1 scatter into W4, biases as an appended all-ones input row, and
the T/V/E components (constant throughout the reference trajectory) into b1.
"""

import os

import numpy as np

DT = 0.1
H = 192
P0 = 96  # first K/M chunk size (H = 2*P0)


def _fold_weights(init_state, times, mean, scale, W1, b1, W2, b2, W3, b3, W4, b4):
    f32 = np.float32
    s0 = np.asarray(init_state, f32)[0]
    mean = np.asarray(mean, f32)
    scale = np.asarray(scale, f32)
    W1 = np.asarray(W1, f32)
    b1 = np.asarray(b1, f32)

    # x = (s - mean)/scale fed to W1  ==>  z1 = s @ (W1/scale) + (b1 - (mean/scale)@W1)
    W1eff = W1 / scale[:, None]
    b1eff = b1 - (mean / scale) @ W1
    # state[0:3] (T, V, E) never changes; fold its layer-1 contribution into the bias
    b1full = b1eff + s0[0:3] @ W1eff[0:3]
    W1v = W1eff[3:7]  # [4, H], input = raw concentrations
    lhs1 = np.concatenate([W1v, b1full[None]], axis=0).astype(f32)  # [5, H]

    lhs2 = np.concatenate([np.asarray(W2, f32), np.asarray(b2, f32)[None]], axis=0)
    lhs3 = np.concatenate([np.asarray(W3, f32), np.asarray(b3, f32)[None]], axis=0)

    # Extended head: 36 outputs (engine APs need 32-aligned partition bases):
    # lanes 0-3 = num increments, lanes 32-35 = den/volume increments
    # (den = [nVF, nVA, nVF, nVB]; the volume triple is lanes [32, 33, 35]).
    # mig = [dNLA, dNK, dVA, dVB] (rates); dN = mig*dt
    A = np.zeros((4, 36), f32)
    A[0, 0] = -1.0  # num CF_LA: -dNLA
    A[0, 1] = +1.0  # num CA_LA: +dNLA
    A[1, 2] = -1.0  # num CF_K:  -dNK
    A[1, 3] = +1.0  # num CB_K:  +dNK
    for j in (32, 34):  # VF lanes: -(dVA+dVB)
        A[2, j] = -1.0
        A[3, j] = -1.0
    A[2, 33] = +1.0  # VA: +dVA
    A[3, 35] = +1.0  # VB: +dVB
    W4e = (np.asarray(W4, f32) @ A) * f32(DT)  # [H, 36]
    b4e = (np.asarray(b4, f32) @ A) * f32(DT)  # [36]
    lhs4 = np.concatenate([W4e, b4e[None]], axis=0).astype(f32)  # [H+1, 36]

    C0 = s0[3:7]
    vden0 = s0[[7, 8, 7, 9]]  # [VF, VA, VF, VB]
    num0 = (C0 * vden0).astype(f32)
    init36 = np.zeros((1, 36), f32)
    init36[0, 0:4] = num0
    init36[0, 32:36] = vden0

    mode = os.environ.get("BMED_MMDT", "f16w")
    wnp = np.float16 if mode == "f16w" else f32
    tensors = {
        "lhs1": np.ascontiguousarray(lhs1.astype(wnp)),
        "l2k0": np.ascontiguousarray(lhs2[0:P0].astype(wnp)),
        "l2k1": np.ascontiguousarray(lhs2[P0:].astype(wnp)),
        "l3k0": np.ascontiguousarray(lhs3[0:P0].astype(wnp)),
        "l3k1": np.ascontiguousarray(lhs3[P0:].astype(wnp)),
        "l4k0": np.ascontiguousarray(lhs4[0:P0].astype(wnp)),
        "l4k1": np.ascontiguousarray(lhs4[P0:].astype(wnp)),
        "init36": np.ascontiguousarray(init36),
        "initC": np.ascontiguousarray(C0[:, None]),              # fp32, traj col 0
        "initX": np.ascontiguousarray(C0[:, None].astype(wnp)),  # MLP input col 0
        "initV": np.ascontiguousarray(vden0[:, None]),
    }
    return tensors, s0


def _build(nc, tc, n_steps, unroll):
    from contextlib import ExitStack

    import concourse.mybir as mybir
    from concourse.bass import ds

    f32 = mybir.dt.float32
    mode = os.environ.get("BMED_MMDT", "f16w")
    wdt = mybir.dt.float16 if mode == "f16w" else f32  # matmul operand dtype
    Kb = P0 + 1  # 97: weight rows P0..191 plus the bias row

    ins = {}
    for name, shape, dt_ in [
        ("lhs1", (5, H), wdt),
        ("l2k0", (P0, H), wdt), ("l2k1", (Kb, H), wdt),
        ("l3k0", (P0, H), wdt), ("l3k1", (Kb, H), wdt),
        ("l4k0", (P0, 36), wdt), ("l4k1", (Kb, 36), wdt),
        ("init36", (1, 36), f32),
        ("initC", (4, 1), f32), ("initX", (4, 1), wdt), ("initV", (4, 1), f32),
    ]:
        ins[name] = nc.dram_tensor(name, shape, dt_, kind="ExternalInput").ap()

    outC = nc.dram_tensor("outC", (4, n_steps + 1), f32, kind="ExternalOutput").ap()
    outV = nc.dram_tensor("outV", (4, n_steps + 1), f32, kind="ExternalOutput").ap()

    n_blocks = n_steps // unroll
    assert n_blocks * unroll == n_steps
    assert unroll % 2 == 0  # x16 ping-pong parity

    stack = ExitStack()
    const = stack.enter_context(tc.tile_pool(name="const", bufs=1))
    psum = stack.enter_context(tc.tile_pool(name="psum", bufs=3, space="PSUM"))
    accp = stack.enter_context(tc.tile_pool(name="accp", bufs=1, space="PSUM"))

    # Weights, resident in SBUF for the whole kernel
    w = {}
    for name, shape in [
        ("lhs1", (5, H)),
        ("l2k0", (P0, H)), ("l2k1", (Kb, H)),
        ("l3k0", (P0, H)), ("l3k1", (Kb, H)),
        ("l4k0", (P0, 36)), ("l4k1", (Kb, 36)),
    ]:
        w[name] = const.tile(list(shape), wdt, name=f"w_{name}", tag=name)
        nc.sync.dma_start(out=w[name][:, :], in_=ins[name])
    w_init36 = const.tile([1, 36], f32, name="w_init36", tag="init36")
    nc.sync.dma_start(out=w_init36[:, :], in_=ins["init36"])

    # Output trajectories (fp32)
    trajC = const.tile([4, n_steps + 1], f32, name="trajC", tag="trajC")
    trajV = const.tile([4, n_steps + 1], f32, name="trajV", tag="trajV")
    nc.sync.dma_start(out=trajC[0:4, 0:1], in_=ins["initC"])
    nc.sync.dma_start(out=trajV[0:4, 0:1], in_=ins["initV"])

    # MLP input ping-pong: [C(4); 1.0] per column; row 4 = constant 1 for bias
    x16 = const.tile([5, 2], wdt, name="x16", tag="x16")
    nc.vector.memset(x16[0:5, :], 1.0)
    nc.sync.dma_start(out=x16[0:4, 0:1], in_=ins["initX"])

    one11 = const.tile([1, 1], f32, name="one11", tag="one11")
    nc.vector.memset(one11[:, :], 1.0)
    rden = const.tile([4, 1], f32, name="rden", tag="rden")
    stage = const.tile([36, 1], f32, name="stage", tag="stage")

    # Hidden tiles [97, 2]; (96, col1) holds the constant 1.0 for bias rows
    hids = []
    for li in range(3):
        ht = const.tile([Kb, 2], wdt, name=f"h{li}", tag=f"h{li}")
        nc.vector.memset(ht[P0:Kb, 1:2], 1.0)
        hids.append(ht)

    # Running sums live in PSUM: lanes 0-3 = num, lanes 32-35 = den/vols
    acc = accp.tile([36, 1], f32, name="acc", tag="acc")
    nc.tensor.matmul(acc[:, :], w_init36[:, :], one11[:, :],
                     start=True, stop=False, skip_group_check=True)

    use_stage = os.environ.get("BMED_STAGE", "0") == "1"
    BANKF = 512  # fp32 elements per PSUM bank (per partition)

    def zpair(z):
        # the two matmul outputs live in adjacent PSUM banks of one tile;
        # view them as [96, 2] with free-stride 512 for a single relu
        return z[:, :].rearrange("p (b c) -> p b c", c=BANKF)[:, :, 0:1]

    def step(u, t_next):
        rhs_x = x16[0:5, (u % 2):(u % 2) + 1]
        z1 = psum.tile([P0, 2 * BANKF], f32, name="z1", tag="z")
        nc.tensor.matmul(z1[:, 0:1], w["lhs1"][:, 0:P0], rhs_x, start=True, stop=True)
        nc.tensor.matmul(z1[:, BANKF:BANKF + 1], w["lhs1"][:, P0:H], rhs_x,
                         start=True, stop=True)
        nc.vector.tensor_relu(hids[0][0:P0, 0:2], zpair(z1))

        for li, (k0, k1) in enumerate((("l2k0", "l2k1"), ("l3k0", "l3k1"))):
            h_in, h_out = hids[li], hids[li + 1]
            z = psum.tile([P0, 2 * BANKF], f32, name="z", tag="z")
            # K1-major order: both K1 matmuls issue as soon as relu_a lands;
            # per-bank accumulation groups keep start/stop independent
            for m_ in range(2):
                nc.tensor.matmul(z[:, m_ * BANKF:m_ * BANKF + 1],
                                 w[k0][:, m_ * P0:(m_ + 1) * P0],
                                 h_in[0:P0, 0:1], start=True, stop=False)
            for m_ in range(2):
                nc.tensor.matmul(z[:, m_ * BANKF:m_ * BANKF + 1],
                                 w[k1][:, m_ * P0:(m_ + 1) * P0],
                                 h_in[0:Kb, 1:2], start=False, stop=True)
            nc.vector.tensor_relu(h_out[0:P0, 0:2], zpair(z))

        h3 = hids[2]
        nc.tensor.matmul(acc[:, :], w["l4k0"][:, :], h3[0:P0, 0:1],
                         start=False, stop=False, skip_group_check=True)
        nc.tensor.matmul(acc[:, :], w["l4k1"][:, :], h3[0:Kb, 1:2],
                         start=False, stop=False, skip_group_check=True)

        # the only acc readers are recip + mult, so the next step's
        # accumulate is not gated by the trajectory writes
        xcol = x16[0:4, ((u + 1) % 2):((u + 1) % 2) + 1]
        nc.vector.reciprocal(rden[:, :], acc[32:36, 0:1])
        nc.vector.tensor_tensor(xcol, acc[0:4, 0:1], rden[:, :],
                                op=mybir.AluOpType.mult)
        # off critical path: trajC = cast of the fp16 x; trajV = den copy
        # (single remaining acc reader on ScalarE)
        nc.scalar.activation(trajC[0:4, t_next], xcol,
                             mybir.ActivationFunctionType.Copy)
        nc.scalar.copy(trajV[0:4, t_next], acc[32:36, 0:1])

    if n_blocks > 1:
        with tc.For_i(0, n_steps, unroll) as i:
            for u in range(unroll):
                step(u, ds(i + u + 1, 1))
    else:
        for u in range(unroll):
            step(u, ds(u + 1, 1))

    if os.environ.get("BMED_DEBUG_H"):
        dbg = nc.dram_tensor("dbgH", (Kb, 6), mybir.dt.float32,
                             kind="ExternalOutput").ap()
        dbg_sb = const.tile([Kb, 6], f32, name="dbg_sb", tag="dbg_sb")
        for li in range(3):
            rows = Kb if li == 2 else P0
            nc.vector.tensor_copy(dbg_sb[0:rows, 2 * li:2 * li + 2],
                                  hids[li][0:rows, 0:2])
        nc.sync.dma_start(out=dbg, in_=dbg_sb[:, :])
    nc.sync.dma_start(out=outC, in_=trajC[0:4, :])
    nc.sync.dma_start(out=outV, in_=trajV[0:4, :])
    stack.close()


def _run_device(tensors, n_steps, unroll, trace=False):
    import concourse.bacc as bacc
    import concourse.bass_utils as bass_utils
    import concourse.tile as tile

    nc = bacc.Bacc("TRN2", target_bir_lowering=False, debug=False,
                   enable_asserts=False, num_devices=8)
    with tile.TileContext(nc) as tc:
        _build(nc, tc, n_steps, unroll)
    nc.compile()

    res = bass_utils.run_bass_kernel_spmd(
        nc, [dict(tensors) for _ in range(8)], core_ids=list(range(8)), trace=trace,
    )
    if getattr(kernel, "_keep_results", False):
        kernel._last_nc = nc
        kernel._last_tensors = tensors
    return res


def kernel(init_state, times, mean, scale, W1, b1, W2, b2, W3, b3, W4, b4):
    times = np.asarray(times)
    n_steps = int(round(float(times[0, -1]) / DT))
    n_steps_env = os.environ.get("BMED_NSTEPS")
    if n_steps_env is not None:
        n_steps = int(n_steps_env)
    unroll = int(os.environ.get("BMED_UNROLL", "20"))
    while n_steps % unroll or unroll % 2:
        unroll -= 1

    tensors, s0 = _fold_weights(init_state, times, mean, scale,
                                W1, b1, W2, b2, W3, b3, W4, b4)
    res = _run_device(tensors, n_steps, unroll,
                      trace=bool(int(os.environ.get("BMED_TRACE", "0"))))
    out = res.results[0]
    C = out["outC"]  # [4, n+1]
    V = out["outV"]  # [4, n+1] = [VF, VA, VF, VB]

    pred = np.empty((n_steps + 1, 10), np.float32)
    pred[:, 0:3] = s0[0:3]
    pred[:, 3:7] = C.T
    pred[:, 7:10] = V[[0, 1, 3], :].T

    measured_indices = np.concatenate([
        np.array([0], dtype=np.int32),
        np.round(times[0] / DT).astype(np.int32),
    ])
    if getattr(kernel, "_keep_results", False):
        kernel._last_res = res
    return pred, measured_indices
